# revision 4
# baseline (speedup 1.0000x reference)
"""
Trainium2 Bass kernel for the Decoder_RNN_Simple problem.

Math (per flat-batch element b, reference semantics):
  hidden0 = tanh(W_z0 @ z0 + b_z0)                       # [256]
  cur0 = 0
  for t in 0..199:
    x = [cur, tps[t]]                                    # [65]
    gx = W_ih @ x + b_ih ; gh = W_hh @ hidden + b_hh     # [768]
    r = sig(gx_r + gh_r); z = sig(gx_z + gh_z)
    n = tanh(gx_n + r * gh_n)
    h' = (1-z)*n + z*h ; pred = W_out @ h' + b_out       # [64]

Mapping (data-parallel over the flat batch of 8192 across 8 cores,
1024 rows per core; transposed [gates, batch] on-chip layout so the
recurrence needs no transposes):

  - cur_t = pred_{t-1} = W_out @ h_t + b_out for t>=1, so the r/z gate
    pre-activations fold into a single matmul with
    W_eff = W_hh + W_ih[:, :64] @ W_out applied to h (exact algebra).
    The n gate keeps xn (from the pred tile, K=64) and hn (from raw
    W_hh) separate since n = tanh(xn + r*hn).
  - All t-dependent bias terms (b_ih + b_hh + W_ih[:,64]*tps[t] (+
    W_ih[:, :64] @ b_out for t>=1)) are precomputed host-side as
    [gate, 200] tables and applied through the ACT bias operand.
"""

import sys

_TRN = "/opt/trn_rl_repo"
if _TRN not in sys.path:
    sys.path.insert(0, _TRN)

import numpy as np

import concourse.bass as bass
import concourse.mybir as mybir
import concourse.tile as tile
from concourse.vector_clock import ScopedClock
from concourse.bass_utils import run_bass_kernel_spmd

N_CORES = 8
LATENT = 128
OUT_DIM = 64
N_GRU = 256
N_TP = 200
B_FULL = 64 * 128
B_LOC = B_FULL // N_CORES  # 1024
HALF = 512
F32 = mybir.dt.float32
AF = mybir.ActivationFunctionType
ALU = mybir.AluOpType


# walrus rejects sem waits carried on the kernel-tail Drain instruction
# ("Too many sync wait commands"); move them onto NOPs, one wait each.
def _patched_drain_and_barrier(self, tick_clock, wait_clock):
    carrier = self.nc.sync.nop()
    wait_clock.add_sem_waits(carrier.ins, ScopedClock({None: tick_clock.global_clock}))
    si = carrier.ins.sync_info
    waits = list(si.on_wait) if si is not None else []
    if len(waits) > 1:
        si.on_wait = waits[:1]
        rest = waits[1:]
        while rest:
            extra = self.nc.sync.nop()
            extra.ins.sync_info = mybir.SyncInfo(on_wait=rest[:1], on_update=[])
            rest = rest[1:]
    self.nc.sync.drain()
    self.nc.all_engine_barrier()
    popped = self.nc._tile_sem_poison_stack.pop()
    assert popped is self._sem_poison
    self.nc.clear_and_free_semaphores(list(self.sems.allocated().values()))
    self.nc.all_engine_barrier()


tile.TileContext._drain_and_barrier = _patched_drain_and_barrier


def _split_waits(nc, maxw=1):
    """This walrus rejects instructions carrying more than a couple of sem
    waits; move the excess onto same-engine NOPs inserted just before."""
    k = 0
    for f in nc.m.functions:
        for bb in f.blocks:
            insts = bb.instructions
            out = []
            changed = False
            for inst in insts:
                si = inst.sync_info
                waits = list(si.on_wait) if si is not None else []
                if len(waits) > maxw:
                    si.on_wait = waits[-maxw:]
                    excess = waits[:-maxw]
                    while excess:
                        chunk, excess = excess[:maxw], excess[maxw:]
                        nop = mybir.InstNoOp(name=f"waitsplit_{k}", ins=[], outs=[])
                        k += 1
                        nop.engine = inst.engine
                        nop.sync_info = mybir.SyncInfo(on_wait=chunk, on_update=[])
                        out.append(nop)
                    changed = True
                out.append(inst)
            if changed:
                bb.instructions = out
    return k


def _build_module():
    nc = bass.Bass("TRN2", target_bir_lowering=False, debug=False, num_devices=N_CORES)

    def inp(name, shape):
        return nc.dram_tensor(name, shape, F32, kind="ExternalInput").ap()

    d = {
        "z0t": inp("z0t", [LATENT, B_LOC]),
        "wz0t": inp("wz0t", [LATENT, N_GRU]),
        "whht1": inp("whht1", [N_GRU, 3 * N_GRU]),  # eff for r,z; raw for n
        "whht0": inp("whht0", [N_GRU, 2 * N_GRU]),  # raw r,z (step 0)
        "wxnt": inp("wxnt", [OUT_DIM, N_GRU]),
        "woutt": inp("woutt", [N_GRU, OUT_DIM]),
        "brz": inp("brz", [2 * N_GRU, N_TP]),
        "bxn": inp("bxn", [N_GRU, N_TP]),
        "bhhn": inp("bhhn", [N_GRU, 1]),
        "bz0": inp("bz0", [N_GRU, 1]),
        "bout": inp("bout", [OUT_DIM, 1]),
    }
    out = nc.dram_tensor("out", [N_TP, OUT_DIM, B_LOC], F32, kind="ExternalOutput").ap()

    with tile.TileContext(nc) as tc:
        _emit(nc, tc, d, out)
    n = _split_waits(nc, maxw=1)
    print(f"[kernel] split {n} excess sem-waits onto NOPs", flush=True)
    return nc


def _emit(nc, tc, d, out):
    with (
        tc.tile_pool(name="const", bufs=1) as cp,
        tc.tile_pool(name="work", bufs=2) as wp,
        tc.tile_pool(name="psum", bufs=3, space="PSUM") as pp,
        tc.tile_pool(name="ppred", bufs=1, space="PSUM") as ppr,
    ):
        def const_tile(name, shape):
            t = cp.tile(shape, F32, tag=name)
            nc.sync.dma_start(t[:], d[name][:])
            return t

        def const_rows(name, shape, r0, tag):
            t = cp.tile(shape, F32, tag=tag)
            nc.sync.dma_start(t[:], d[name][r0 : r0 + shape[0], :])
            return t

        wz0 = const_tile("wz0t", [LATENT, N_GRU])
        whh1 = [const_rows("whht1", [128, 3 * N_GRU], 128 * k, f"whh1_{k}") for k in range(2)]
        whh0 = [const_rows("whht0", [128, 2 * N_GRU], 128 * k, f"whh0_{k}") for k in range(2)]
        wxn = const_tile("wxnt", [OUT_DIM, N_GRU])
        wout = [const_rows("woutt", [128, OUT_DIM], 128 * k, f"wout_{k}") for k in range(2)]
        brz = [const_rows("brz", [128, N_TP], 128 * g, f"brz_{g}") for g in range(4)]
        bxn = [const_rows("bxn", [128, N_TP], 128 * c, f"bxn_{c}") for c in range(2)]
        bhhn = [const_rows("bhhn", [128, 1], 128 * c, f"bhhn_{c}") for c in range(2)]
        bz0 = [const_rows("bz0", [128, 1], 128 * c, f"bz0_{c}") for c in range(2)]
        bout = const_tile("bout", [OUT_DIM, 1])

        # ---- initial hidden: h = tanh(Wz0 @ z0T + b_z0), [256, B] as 2 chunks
        z0sb = wp.tile([LATENT, B_LOC], F32, tag="z0")
        nc.sync.dma_start(z0sb[:], d["z0t"][:])
        h = []
        for c in range(2):
            p = pp.tile([128, B_LOC], F32, tag="ps")
            for hf in range(2):
                hs = slice(hf * HALF, (hf + 1) * HALF)
                nc.tensor.matmul(
                    p[:, hs], wz0[:, c * 128 : (c + 1) * 128], z0sb[:, hs],
                    start=True, stop=True,
                )
            hc = wp.tile([128, B_LOC], F32, tag=f"h{c}")
            nc.scalar.activation(hc[:], p[:], AF.Tanh, bias=bz0[c][:, 0:1])
            h.append(hc)

        pred = None
        for t in range(N_TP):
            first = t == 0
            wk = whh0 if first else whh1

            # r and z gate chunks: accumulate (W_eff @ h) in PSUM, sigmoid out
            sig = []
            for g in range(4):  # r0 r1 z0 z1
                p = pp.tile([128, B_LOC], F32, tag="ps")
                col = slice(g * 128, (g + 1) * 128)
                for hf in range(2):
                    hs = slice(hf * HALF, (hf + 1) * HALF)
                    nc.tensor.matmul(p[:, hs], wk[0][:, col], h[0][:, hs],
                                     start=True, stop=False)
                    nc.tensor.matmul(p[:, hs], wk[1][:, col], h[1][:, hs],
                                     start=False, stop=True)
                s = wp.tile([128, B_LOC], F32, tag=f"sig{g}")
                nc.scalar.activation(s[:], p[:], AF.Sigmoid, bias=brz[g][:, t : t + 1])
                sig.append(s)
            r, zg = sig[:2], sig[2:]

            # n gate: hn (raw W_hh) and xn (from pred, K=64) kept separate
            phn = []
            for c in range(2):
                col = slice(512 + c * 128, 512 + (c + 1) * 128)
                p = pp.tile([128, B_LOC], F32, tag="ps")
                for hf in range(2):
                    hs = slice(hf * HALF, (hf + 1) * HALF)
                    nc.tensor.matmul(p[:, hs], whh1[0][:, col], h[0][:, hs],
                                     start=True, stop=False)
                    nc.tensor.matmul(p[:, hs], whh1[1][:, col], h[1][:, hs],
                                     start=False, stop=True)
                phn.append(p)
            pxn = []
            if not first:
                for c in range(2):
                    col = slice(c * 128, (c + 1) * 128)
                    p = pp.tile([128, B_LOC], F32, tag="ps")
                    for hf in range(2):
                        hs = slice(hf * HALF, (hf + 1) * HALF)
                        nc.tensor.matmul(p[:, hs], wxn[:, col], pred[:, hs],
                                         start=True, stop=True)
                    pxn.append(p)

            h_new = []
            for c in range(2):
                # t1 = (hn + b_hhn) * r   (one fused DVE op)
                t1 = wp.tile([128, B_LOC], F32, tag=f"t1_{c}")
                nc.vector.scalar_tensor_tensor(
                    t1[:], phn[c][:], bhhn[c][:, 0:1], r[c][:], ALU.add, ALU.mult
                )
                if first:
                    t2 = t1
                else:
                    t2 = wp.tile([128, B_LOC], F32, tag=f"t2_{c}")
                    nc.vector.tensor_tensor(t2[:], t1[:], pxn[c][:], ALU.add)
                n = wp.tile([128, B_LOC], F32, tag=f"n_{c}")
                nc.scalar.activation(n[:], t2[:], AF.Tanh, bias=bxn[c][:, t : t + 1])
                # h' = n + z*(h-n)
                dt_ = wp.tile([128, B_LOC], F32, tag=f"d_{c}")
                nc.gpsimd.tensor_sub(dt_[:], h[c][:], n[:])
                e = wp.tile([128, B_LOC], F32, tag=f"e_{c}")
                nc.vector.tensor_mul(e[:], zg[c][:], dt_[:])
                hc = wp.tile([128, B_LOC], F32, tag=f"h{c}")
                nc.vector.tensor_add(hc[:], e[:], n[:])
                h_new.append(hc)
            h = h_new

            # pred = W_out @ h' + b_out  -> DRAM out[t], and rhs for next xn
            p = ppr.tile([OUT_DIM, B_LOC], F32, tag="pp")
            for hf in range(2):
                hs = slice(hf * HALF, (hf + 1) * HALF)
                nc.tensor.matmul(p[:, hs], wout[0][:, :], h[0][:, hs],
                                 start=True, stop=False)
                nc.tensor.matmul(p[:, hs], wout[1][:, :], h[1][:, hs],
                                 start=False, stop=True)
            pred = wp.tile([OUT_DIM, B_LOC], F32, tag="pred")
            nc.scalar.activation(pred[:], p[:], AF.Identity, bias=bout[:, 0:1])
            nc.sync.dma_start(out[t], pred[:])


_CACHE = {}


def _prep_host(z0, tps_to_pred, W_z0, b_z0, W_ih, b_ih, W_hh, b_hh, W_out, b_out):
    f = np.float32
    z0 = np.asarray(z0, f)
    tps = np.asarray(tps_to_pred, f)
    W_z0, b_z0 = np.asarray(W_z0, f), np.asarray(b_z0, f)
    W_ih, b_ih = np.asarray(W_ih, f), np.asarray(b_ih, f)
    W_hh, b_hh = np.asarray(W_hh, f), np.asarray(b_hh, f)
    W_out, b_out = np.asarray(W_out, f), np.asarray(b_out, f)

    Wihp = W_ih[:, :OUT_DIM]  # [768, 64]
    wt = W_ih[:, OUT_DIM]  # [768]
    G2 = 2 * N_GRU
    Weff_rz = W_hh[:G2] + Wihp[:G2] @ W_out  # [512, 256]
    whht1 = np.ascontiguousarray(
        np.concatenate([Weff_rz, W_hh[G2:]], axis=0).T
    )  # [256, 768]
    whht0 = np.ascontiguousarray(W_hh[:G2].T)  # [256, 512]
    wxnt = np.ascontiguousarray(Wihp[G2:].T)  # [64, 256]
    woutt = np.ascontiguousarray(W_out.T)  # [256, 64]

    cb = Wihp @ b_out  # [768]
    bias_all = b_ih[:, None] + wt[:, None] * tps[None, :]  # [768, 200]
    brz = bias_all[:G2] + b_hh[:G2, None]
    brz[:, 1:] += cb[:G2, None]
    bxn = bias_all[G2:].copy()
    bxn[:, 1:] += cb[G2:, None]

    shared = {
        "wz0t": np.ascontiguousarray(W_z0.T),
        "whht1": whht1,
        "whht0": whht0,
        "wxnt": wxnt,
        "woutt": woutt,
        "brz": np.ascontiguousarray(brz, f),
        "bxn": np.ascontiguousarray(bxn, f),
        "bhhn": np.ascontiguousarray(b_hh[G2:].reshape(N_GRU, 1)),
        "bz0": np.ascontiguousarray(b_z0.reshape(N_GRU, 1)),
        "bout": np.ascontiguousarray(b_out.reshape(OUT_DIM, 1)),
    }
    z0f = z0.reshape(B_FULL, LATENT)
    in_maps = []
    for i in range(N_CORES):
        m = dict(shared)
        m["z0t"] = np.ascontiguousarray(z0f[i * B_LOC : (i + 1) * B_LOC].T)
        in_maps.append(m)
    return in_maps


def _run(in_maps, **spmd_kwargs):
    if "nc" not in _CACHE:
        _CACHE["nc"] = _build_module()
    return run_bass_kernel_spmd(_CACHE["nc"], in_maps, list(range(N_CORES)), **spmd_kwargs)


def _gather(res):
    outp = np.empty((B_FULL, N_TP, OUT_DIM), np.float32)
    for i in range(N_CORES):
        o = res.results[i]["out"]  # [200, 64, 1024]
        outp[i * B_LOC : (i + 1) * B_LOC] = np.asarray(o).transpose(2, 0, 1)
    return outp.reshape(64, 128, N_TP, OUT_DIM)


def kernel(**inputs):
    in_maps = _prep_host(**inputs)
    res = _run(in_maps)
    return _gather(res)


def kernel_profiled(**inputs):
    """Like kernel(), but requests an NTFF trace; returns (output, results)."""
    in_maps = _prep_host(**inputs)
    res = _run(in_maps, trace=True)
    return _gather(res), res


# revision 7
# speedup vs baseline: 13.4689x; 13.4689x over previous
"""
Trainium2 Bass kernel for the Decoder_RNN_Simple problem.

Math (per flat-batch element b, reference semantics):
  hidden0 = tanh(W_z0 @ z0 + b_z0)                       # [256]
  cur0 = 0
  for t in 0..199:
    x = [cur, tps[t]]                                    # [65]
    gx = W_ih @ x + b_ih ; gh = W_hh @ hidden + b_hh     # [768]
    r = sig(gx_r + gh_r); z = sig(gx_z + gh_z)
    n = tanh(gx_n + r * gh_n)
    h' = (1-z)*n + z*h ; pred = W_out @ h' + b_out       # [64]

Mapping (data-parallel over the flat batch of 8192 across 8 cores,
1024 rows per core; transposed [gates, batch] on-chip layout so the
recurrence needs no transposes):

  - cur_t = pred_{t-1} = W_out @ h_t + b_out for t>=1, so the r/z gate
    pre-activations fold into a single matmul with
    W_eff = W_hh + W_ih[:, :64] @ W_out applied to h (exact algebra).
    The n gate keeps xn (from the pred tile, K=64) and hn (from raw
    W_hh) separate since n = tanh(xn + r*hn).
  - All t-dependent bias terms (b_ih + b_hh + W_ih[:,64]*tps[t] (+
    W_ih[:, :64] @ b_out for t>=1)) are precomputed host-side as
    [gate, 200] tables and applied through the ACT bias operand.
"""

import sys

_TRN = "/opt/trn_rl_repo"
if _TRN not in sys.path:
    sys.path.insert(0, _TRN)

import numpy as np

import concourse.bass as bass
import concourse.mybir as mybir
import concourse.tile as tile
from concourse.vector_clock import ScopedClock
from concourse.bass_utils import run_bass_kernel_spmd

N_CORES = 8
LATENT = 128
OUT_DIM = 64
N_GRU = 256
N_TP = 200
B_FULL = 64 * 128
B_LOC = B_FULL // N_CORES  # 1024
HALF = 512
F32 = mybir.dt.float32
AF = mybir.ActivationFunctionType
ALU = mybir.AluOpType


# walrus rejects sem waits carried on the kernel-tail Drain instruction
# ("Too many sync wait commands"); move them onto NOPs, one wait each.
def _patched_drain_and_barrier(self, tick_clock, wait_clock):
    carrier = self.nc.sync.nop()
    wait_clock.add_sem_waits(carrier.ins, ScopedClock({None: tick_clock.global_clock}))
    si = carrier.ins.sync_info
    waits = list(si.on_wait) if si is not None else []
    if len(waits) > 1:
        si.on_wait = waits[:1]
        rest = waits[1:]
        while rest:
            extra = self.nc.sync.nop()
            extra.ins.sync_info = mybir.SyncInfo(on_wait=rest[:1], on_update=[])
            rest = rest[1:]
    self.nc.sync.drain()
    self.nc.all_engine_barrier()
    popped = self.nc._tile_sem_poison_stack.pop()
    assert popped is self._sem_poison
    self.nc.clear_and_free_semaphores(list(self.sems.allocated().values()))
    self.nc.all_engine_barrier()


tile.TileContext._drain_and_barrier = _patched_drain_and_barrier


def _split_waits(nc, maxw=1):
    """This walrus rejects instructions carrying more than a couple of sem
    waits; move the excess onto same-engine NOPs inserted just before."""
    k = 0
    for f in nc.m.functions:
        for bb in f.blocks:
            insts = bb.instructions
            out = []
            changed = False
            for inst in insts:
                si = inst.sync_info
                waits = list(si.on_wait) if si is not None else []
                if len(waits) > maxw:
                    si.on_wait = waits[-maxw:]
                    excess = waits[:-maxw]
                    while excess:
                        chunk, excess = excess[:maxw], excess[maxw:]
                        nop = mybir.InstNoOp(name=f"waitsplit_{k}", ins=[], outs=[])
                        k += 1
                        nop.engine = inst.engine
                        nop.sync_info = mybir.SyncInfo(on_wait=chunk, on_update=[])
                        out.append(nop)
                    changed = True
                out.append(inst)
            if changed:
                bb.instructions = out
    return k


def _build_module(repeat=1):
    nc = bass.Bass("TRN2", target_bir_lowering=False, debug=False, num_devices=N_CORES)

    def inp(name, shape):
        return nc.dram_tensor(name, shape, F32, kind="ExternalInput").ap()

    d = {
        "z0t": inp("z0t", [LATENT, B_LOC]),
        "wz0t": inp("wz0t", [LATENT, N_GRU]),
        "whht1": inp("whht1", [N_GRU, 3 * N_GRU]),  # eff for r,z; raw for n
        "whht0": inp("whht0", [N_GRU, 2 * N_GRU]),  # raw r,z (step 0)
        "wxnt": inp("wxnt", [OUT_DIM, N_GRU]),
        "woutt": inp("woutt", [N_GRU, OUT_DIM]),
        "brz": inp("brz", [2 * N_GRU, N_TP]),
        "bxn": inp("bxn", [N_GRU, N_TP]),
        "bhhn": inp("bhhn", [N_GRU, 1]),
        "bz0": inp("bz0", [N_GRU, 1]),
        "bout": inp("bout", [OUT_DIM, 1]),
    }
    out = nc.dram_tensor("out", [N_TP, OUT_DIM, B_LOC], F32, kind="ExternalOutput").ap()

    with tile.TileContext(nc) as tc:
        for _ in range(repeat):
            _emit(nc, tc, d, out)
    n = _split_waits(nc, maxw=1)
    print(f"[kernel] split {n} excess sem-waits onto NOPs", flush=True)
    return nc


def _emit(nc, tc, d, out):
    with (
        tc.tile_pool(name="const", bufs=1) as cp,
        tc.tile_pool(name="work", bufs=2) as wp,
        tc.tile_pool(name="psum", bufs=3, space="PSUM") as pp,
        tc.tile_pool(name="ppred", bufs=1, space="PSUM") as ppr,
    ):
        def const_tile(name, shape):
            t = cp.tile(shape, F32, tag=name)
            nc.sync.dma_start(t[:], d[name][:])
            return t

        def const_rows(name, shape, r0, tag):
            t = cp.tile(shape, F32, tag=tag)
            nc.sync.dma_start(t[:], d[name][r0 : r0 + shape[0], :])
            return t

        wz0 = const_tile("wz0t", [LATENT, N_GRU])
        whh1 = [const_rows("whht1", [128, 3 * N_GRU], 128 * k, f"whh1_{k}") for k in range(2)]
        whh0 = [const_rows("whht0", [128, 2 * N_GRU], 128 * k, f"whh0_{k}") for k in range(2)]
        wxn = const_tile("wxnt", [OUT_DIM, N_GRU])
        wout = [const_rows("woutt", [128, OUT_DIM], 128 * k, f"wout_{k}") for k in range(2)]
        brz = [const_rows("brz", [128, N_TP], 128 * g, f"brz_{g}") for g in range(4)]
        bxn = [const_rows("bxn", [128, N_TP], 128 * c, f"bxn_{c}") for c in range(2)]
        bhhn = [const_rows("bhhn", [128, 1], 128 * c, f"bhhn_{c}") for c in range(2)]
        bz0 = [const_rows("bz0", [128, 1], 128 * c, f"bz0_{c}") for c in range(2)]
        bout = const_tile("bout", [OUT_DIM, 1])

        # ---- initial hidden: h = tanh(Wz0 @ z0T + b_z0), [256, B] as 2 chunks
        z0sb = wp.tile([LATENT, B_LOC], F32, tag="z0")
        nc.sync.dma_start(z0sb[:], d["z0t"][:])
        h = []
        for c in range(2):
            p = pp.tile([128, B_LOC], F32, tag="ps")
            for hf in range(2):
                hs = slice(hf * HALF, (hf + 1) * HALF)
                nc.tensor.matmul(
                    p[:, hs], wz0[:, c * 128 : (c + 1) * 128], z0sb[:, hs],
                    start=True, stop=True,
                )
            hc = wp.tile([128, B_LOC], F32, tag=f"h{c}")
            nc.scalar.activation(hc[:], p[:], AF.Tanh, bias=bz0[c][:, 0:1])
            h.append(hc)

        pred = None
        for t in range(N_TP):
            first = t == 0
            wk = whh0 if first else whh1

            # r and z gate chunks: accumulate (W_eff @ h) in PSUM, sigmoid out
            sig = []
            for g in range(4):  # r0 r1 z0 z1
                p = pp.tile([128, B_LOC], F32, tag="ps")
                col = slice(g * 128, (g + 1) * 128)
                for hf in range(2):
                    hs = slice(hf * HALF, (hf + 1) * HALF)
                    nc.tensor.matmul(p[:, hs], wk[0][:, col], h[0][:, hs],
                                     start=True, stop=False)
                    nc.tensor.matmul(p[:, hs], wk[1][:, col], h[1][:, hs],
                                     start=False, stop=True)
                s = wp.tile([128, B_LOC], F32, tag=f"sig{g}")
                nc.scalar.activation(s[:], p[:], AF.Sigmoid, bias=brz[g][:, t : t + 1])
                sig.append(s)
            r, zg = sig[:2], sig[2:]

            # n gate: hn (raw W_hh) and xn (from pred, K=64) kept separate
            phn = []
            for c in range(2):
                col = slice(512 + c * 128, 512 + (c + 1) * 128)
                p = pp.tile([128, B_LOC], F32, tag="ps")
                for hf in range(2):
                    hs = slice(hf * HALF, (hf + 1) * HALF)
                    nc.tensor.matmul(p[:, hs], whh1[0][:, col], h[0][:, hs],
                                     start=True, stop=False)
                    nc.tensor.matmul(p[:, hs], whh1[1][:, col], h[1][:, hs],
                                     start=False, stop=True)
                phn.append(p)
            pxn = []
            if not first:
                for c in range(2):
                    col = slice(c * 128, (c + 1) * 128)
                    p = pp.tile([128, B_LOC], F32, tag="ps")
                    for hf in range(2):
                        hs = slice(hf * HALF, (hf + 1) * HALF)
                        nc.tensor.matmul(p[:, hs], wxn[:, col], pred[:, hs],
                                         start=True, stop=True)
                    pxn.append(p)

            h_new = []
            for c in range(2):
                # t1 = (hn + b_hhn) * r   (one fused DVE op)
                t1 = wp.tile([128, B_LOC], F32, tag=f"t1_{c}")
                nc.vector.scalar_tensor_tensor(
                    t1[:], phn[c][:], bhhn[c][:, 0:1], r[c][:], ALU.add, ALU.mult
                )
                if first:
                    t2 = t1
                else:
                    t2 = wp.tile([128, B_LOC], F32, tag=f"t2_{c}")
                    nc.vector.tensor_tensor(t2[:], t1[:], pxn[c][:], ALU.add)
                n = wp.tile([128, B_LOC], F32, tag=f"n_{c}")
                nc.scalar.activation(n[:], t2[:], AF.Tanh, bias=bxn[c][:, t : t + 1])
                # h' = n + z*(h-n)
                dt_ = wp.tile([128, B_LOC], F32, tag=f"d_{c}")
                nc.gpsimd.tensor_sub(dt_[:], h[c][:], n[:])
                e = wp.tile([128, B_LOC], F32, tag=f"e_{c}")
                nc.vector.tensor_mul(e[:], zg[c][:], dt_[:])
                hc = wp.tile([128, B_LOC], F32, tag=f"h{c}")
                nc.vector.tensor_add(hc[:], e[:], n[:])
                h_new.append(hc)
            h = h_new

            # pred = W_out @ h' + b_out  -> DRAM out[t], and rhs for next xn
            p = ppr.tile([OUT_DIM, B_LOC], F32, tag="pp")
            for hf in range(2):
                hs = slice(hf * HALF, (hf + 1) * HALF)
                nc.tensor.matmul(p[:, hs], wout[0][:, :], h[0][:, hs],
                                 start=True, stop=False)
                nc.tensor.matmul(p[:, hs], wout[1][:, :], h[1][:, hs],
                                 start=False, stop=True)
            pred = wp.tile([OUT_DIM, B_LOC], F32, tag="pred")
            nc.scalar.activation(pred[:], p[:], AF.Identity, bias=bout[:, 0:1])
            nc.sync.dma_start(out[t], pred[:])


_CACHE = {}


def _prep_host(z0, tps_to_pred, W_z0, b_z0, W_ih, b_ih, W_hh, b_hh, W_out, b_out):
    f = np.float32
    z0 = np.asarray(z0, f)
    tps = np.asarray(tps_to_pred, f)
    W_z0, b_z0 = np.asarray(W_z0, f), np.asarray(b_z0, f)
    W_ih, b_ih = np.asarray(W_ih, f), np.asarray(b_ih, f)
    W_hh, b_hh = np.asarray(W_hh, f), np.asarray(b_hh, f)
    W_out, b_out = np.asarray(W_out, f), np.asarray(b_out, f)

    Wihp = W_ih[:, :OUT_DIM]  # [768, 64]
    wt = W_ih[:, OUT_DIM]  # [768]
    G2 = 2 * N_GRU
    Weff_rz = W_hh[:G2] + Wihp[:G2] @ W_out  # [512, 256]
    whht1 = np.ascontiguousarray(
        np.concatenate([Weff_rz, W_hh[G2:]], axis=0).T
    )  # [256, 768]
    whht0 = np.ascontiguousarray(W_hh[:G2].T)  # [256, 512]
    wxnt = np.ascontiguousarray(Wihp[G2:].T)  # [64, 256]
    woutt = np.ascontiguousarray(W_out.T)  # [256, 64]

    cb = Wihp @ b_out  # [768]
    bias_all = b_ih[:, None] + wt[:, None] * tps[None, :]  # [768, 200]
    brz = bias_all[:G2] + b_hh[:G2, None]
    brz[:, 1:] += cb[:G2, None]
    bxn = bias_all[G2:].copy()
    bxn[:, 1:] += cb[G2:, None]

    shared = {
        "wz0t": np.ascontiguousarray(W_z0.T),
        "whht1": whht1,
        "whht0": whht0,
        "wxnt": wxnt,
        "woutt": woutt,
        "brz": np.ascontiguousarray(brz, f),
        "bxn": np.ascontiguousarray(bxn, f),
        "bhhn": np.ascontiguousarray(b_hh[G2:].reshape(N_GRU, 1)),
        "bz0": np.ascontiguousarray(b_z0.reshape(N_GRU, 1)),
        "bout": np.ascontiguousarray(b_out.reshape(OUT_DIM, 1)),
    }
    z0f = z0.reshape(B_FULL, LATENT)
    in_maps = []
    for i in range(N_CORES):
        m = dict(shared)
        m["z0t"] = np.ascontiguousarray(z0f[i * B_LOC : (i + 1) * B_LOC].T)
        in_maps.append(m)
    return in_maps


def _run(in_maps, repeat=1, **spmd_kwargs):
    key = f"nc{repeat}"
    if key not in _CACHE:
        _CACHE[key] = _build_module(repeat)
    return run_bass_kernel_spmd(_CACHE[key], in_maps, list(range(N_CORES)), **spmd_kwargs)


def _gather(res):
    outp = np.empty((B_FULL, N_TP, OUT_DIM), np.float32)
    for i in range(N_CORES):
        o = res.results[i]["out"]  # [200, 64, 1024]
        outp[i * B_LOC : (i + 1) * B_LOC] = np.asarray(o).transpose(2, 0, 1)
    return outp.reshape(64, 128, N_TP, OUT_DIM)


def kernel(**inputs):
    in_maps = _prep_host(**inputs)
    res = _run(in_maps)
    return _gather(res)


def kernel_profiled(**inputs):
    """Like kernel(), but requests an NTFF trace; returns (output, results)."""
    in_maps = _prep_host(**inputs)
    res = _run(in_maps, trace=True)
    return _gather(res), res


# revision 9
# speedup vs baseline: 2685.1689x; 199.3611x over previous
"""
Trainium2 Bass kernel for the Decoder_RNN_Simple problem.

Math (per flat-batch element b, reference semantics):
  hidden0 = tanh(W_z0 @ z0 + b_z0)                       # [256]
  cur0 = 0
  for t in 0..199:
    x = [cur, tps[t]]                                    # [65]
    gx = W_ih @ x + b_ih ; gh = W_hh @ hidden + b_hh     # [768]
    r = sig(gx_r + gh_r); z = sig(gx_z + gh_z)
    n = tanh(gx_n + r * gh_n)
    h' = (1-z)*n + z*h ; pred = W_out @ h' + b_out       # [64]

Mapping (data-parallel over the flat batch of 8192 across 8 cores,
1024 rows per core; transposed [gates, batch] on-chip layout so the
recurrence needs no transposes):

  - cur_t = pred_{t-1} = W_out @ h_t + b_out for t>=1, so the r/z gate
    pre-activations fold into a single matmul with
    W_eff = W_hh + W_ih[:, :64] @ W_out applied to h (exact algebra).
    The n gate keeps xn (from the pred tile, K=64) and hn (from raw
    W_hh) separate since n = tanh(xn + r*hn).
  - All t-dependent bias terms (b_ih + b_hh + W_ih[:,64]*tps[t] (+
    W_ih[:, :64] @ b_out for t>=1)) are precomputed host-side as
    [gate, 200] tables and applied through the ACT bias operand.
"""

import sys

_TRN = "/opt/trn_rl_repo"
if _TRN not in sys.path:
    sys.path.insert(0, _TRN)

import numpy as np

import concourse.bass as bass
import concourse.mybir as mybir
import concourse.tile as tile
from concourse.vector_clock import ScopedClock
from concourse.bass_utils import run_bass_kernel_spmd

N_CORES = 8
LATENT = 128
OUT_DIM = 64
N_GRU = 256
N_TP = 200
B_FULL = 64 * 128
B_LOC = B_FULL // N_CORES  # 1024
HALF = 512
F32 = mybir.dt.float32
AF = mybir.ActivationFunctionType
ALU = mybir.AluOpType


# walrus rejects sem waits carried on the kernel-tail Drain instruction
# ("Too many sync wait commands"); move them onto NOPs, one wait each.
def _patched_drain_and_barrier(self, tick_clock, wait_clock):
    carrier = self.nc.sync.nop()
    wait_clock.add_sem_waits(carrier.ins, ScopedClock({None: tick_clock.global_clock}))
    si = carrier.ins.sync_info
    waits = list(si.on_wait) if si is not None else []
    if len(waits) > 1:
        si.on_wait = waits[:1]
        rest = waits[1:]
        while rest:
            extra = self.nc.sync.nop()
            extra.ins.sync_info = mybir.SyncInfo(on_wait=rest[:1], on_update=[])
            rest = rest[1:]
    self.nc.sync.drain()
    self.nc.all_engine_barrier()
    popped = self.nc._tile_sem_poison_stack.pop()
    assert popped is self._sem_poison
    self.nc.clear_and_free_semaphores(list(self.sems.allocated().values()))
    self.nc.all_engine_barrier()


tile.TileContext._drain_and_barrier = _patched_drain_and_barrier


def _split_waits(nc, maxw=1):
    """This walrus rejects instructions carrying more than a couple of sem
    waits; move the excess onto same-engine NOPs inserted just before."""
    k = 0
    for f in nc.m.functions:
        for bb in f.blocks:
            insts = bb.instructions
            out = []
            changed = False
            for inst in insts:
                si = inst.sync_info
                waits = list(si.on_wait) if si is not None else []
                if len(waits) > maxw:
                    si.on_wait = waits[-maxw:]
                    excess = waits[:-maxw]
                    while excess:
                        chunk, excess = excess[:maxw], excess[maxw:]
                        nop = mybir.InstNoOp(name=f"waitsplit_{k}", ins=[], outs=[])
                        k += 1
                        nop.engine = inst.engine
                        nop.sync_info = mybir.SyncInfo(on_wait=chunk, on_update=[])
                        out.append(nop)
                    changed = True
                out.append(inst)
            if changed:
                bb.instructions = out
    return k


def _build_module(repeat=1):
    nc = bass.Bass("TRN2", target_bir_lowering=False, debug=False, num_devices=N_CORES)

    def inp(name, shape):
        return nc.dram_tensor(name, shape, F32, kind="ExternalInput").ap()

    d = {
        "z0t": inp("z0t", [LATENT, B_LOC]),
        "wz0t": inp("wz0t", [LATENT, N_GRU]),
        "whht1": inp("whht1", [N_GRU, 3 * N_GRU]),  # eff for r,z; raw for n
        "whht0": inp("whht0", [N_GRU, 2 * N_GRU]),  # raw r,z (step 0)
        "wxnt": inp("wxnt", [OUT_DIM, N_GRU]),
        "woutt": inp("woutt", [N_GRU, OUT_DIM]),
        "brz": inp("brz", [2 * N_GRU, N_TP]),
        "bxn": inp("bxn", [N_GRU, N_TP]),
        "bhhn": inp("bhhn", [N_GRU, 1]),
        "bz0": inp("bz0", [N_GRU, 1]),
        "bout": inp("bout", [OUT_DIM, 1]),
    }
    out = nc.dram_tensor("out", [N_TP, OUT_DIM, B_LOC], F32, kind="ExternalOutput").ap()

    with tile.TileContext(nc) as tc:
        for _ in range(repeat):
            _emit(nc, tc, d, out)
    n = _split_waits(nc, maxw=1)
    print(f"[kernel] split {n} excess sem-waits onto NOPs", flush=True)
    return nc


NSTREAM = 2  # independent sub-batch recurrences per core (hides serial chain)


def _emit(nc, tc, d, out):
    SW = B_LOC // NSTREAM  # batch columns per stream
    ns = NSTREAM
    with (
        tc.tile_pool(name="const", bufs=1) as cp,
        tc.tile_pool(name="work", bufs=2) as wp,
        tc.tile_pool(name="psum", bufs=6, space="PSUM") as pp,
        tc.tile_pool(name="ppred", bufs=2, space="PSUM") as ppr,
    ):
        def const_tile(name, shape):
            t = cp.tile(shape, F32, tag=name)
            nc.sync.dma_start(t[:], d[name][:])
            return t

        def const_rows(name, shape, r0, tag):
            t = cp.tile(shape, F32, tag=tag)
            nc.sync.dma_start(t[:], d[name][r0 : r0 + shape[0], :])
            return t

        wz0 = const_tile("wz0t", [LATENT, N_GRU])
        whh1 = [const_rows("whht1", [128, 3 * N_GRU], 128 * k, f"whh1_{k}") for k in range(2)]
        whh0 = [const_rows("whht0", [128, 2 * N_GRU], 128 * k, f"whh0_{k}") for k in range(2)]
        wxn = const_tile("wxnt", [OUT_DIM, N_GRU])
        wout = [const_rows("woutt", [128, OUT_DIM], 128 * k, f"wout_{k}") for k in range(2)]
        brz = [const_rows("brz", [128, N_TP], 128 * g, f"brz_{g}") for g in range(4)]
        bxn = [const_rows("bxn", [128, N_TP], 128 * c, f"bxn_{c}") for c in range(2)]
        bhhn = [const_rows("bhhn", [128, 1], 128 * c, f"bhhn_{c}") for c in range(2)]
        bz0 = [const_rows("bz0", [128, 1], 128 * c, f"bz0_{c}") for c in range(2)]
        bout = const_tile("bout", [OUT_DIM, 1])

        # ---- initial hidden: h = tanh(Wz0 @ z0T + b_z0), [256, B] as 2 chunks
        z0sb = wp.tile([LATENT, B_LOC], F32, tag="z0")
        nc.sync.dma_start(z0sb[:], d["z0t"][:])
        # h[s][c]: stream s, gate chunk c -> [128, SW]
        h = [[None, None] for _ in range(ns)]
        for s in range(ns):
            bs = slice(s * SW, (s + 1) * SW)
            for c in range(2):
                p = pp.tile([128, SW], F32, tag="ps")
                nc.tensor.matmul(p[:], wz0[:, c * 128 : (c + 1) * 128], z0sb[:, bs],
                                 start=True, stop=True)
                hc = wp.tile([128, SW], F32, tag=f"h{c}_{s}")
                nc.scalar.activation(hc[:], p[:], AF.Tanh, bias=bz0[c][:, 0:1])
                h[s][c] = hc

        pred = [None] * ns
        for t in range(N_TP):
            first = t == 0
            wk = whh0 if first else whh1

            # r and z gate chunks: accumulate (W_eff @ h) in PSUM, sigmoid out
            sig = [[None] * 4 for _ in range(ns)]
            for g in range(4):  # r0 r1 z0 z1
                col = slice(g * 128, (g + 1) * 128)
                for s in range(ns):
                    p = pp.tile([128, SW], F32, tag="ps")
                    nc.tensor.matmul(p[:], wk[0][:, col], h[s][0][:],
                                     start=True, stop=False)
                    nc.tensor.matmul(p[:], wk[1][:, col], h[s][1][:],
                                     start=False, stop=True)
                    sg = wp.tile([128, SW], F32, tag=f"sig{g}_{s}")
                    nc.scalar.activation(sg[:], p[:], AF.Sigmoid,
                                         bias=brz[g][:, t : t + 1])
                    sig[s][g] = sg

            # n gate: hn (raw W_hh) and xn (from pred, K=64) kept separate
            phn = [[None, None] for _ in range(ns)]
            pxn = [[None, None] for _ in range(ns)]
            for c in range(2):
                col = slice(512 + c * 128, 512 + (c + 1) * 128)
                xcol = slice(c * 128, (c + 1) * 128)
                for s in range(ns):
                    p = pp.tile([128, SW], F32, tag="ps")
                    nc.tensor.matmul(p[:], whh1[0][:, col], h[s][0][:],
                                     start=True, stop=False)
                    nc.tensor.matmul(p[:], whh1[1][:, col], h[s][1][:],
                                     start=False, stop=True)
                    phn[s][c] = p
                    if not first:
                        px = pp.tile([128, SW], F32, tag="ps")
                        nc.tensor.matmul(px[:], wxn[:, xcol], pred[s][:],
                                         start=True, stop=True)
                        pxn[s][c] = px

            h_new = [[None, None] for _ in range(ns)]
            for c in range(2):
                for s in range(ns):
                    # t1 = (hn + b_hhn) * r   (one fused DVE op)
                    t1 = wp.tile([128, SW], F32, tag=f"t1_{c}_{s}")
                    nc.vector.scalar_tensor_tensor(
                        t1[:], phn[s][c][:], bhhn[c][:, 0:1], sig[s][c][:],
                        ALU.add, ALU.mult,
                    )
                    if first:
                        t2 = t1
                    else:
                        t2 = wp.tile([128, SW], F32, tag=f"t2_{c}_{s}")
                        nc.vector.tensor_tensor(t2[:], t1[:], pxn[s][c][:], ALU.add)
                    n = wp.tile([128, SW], F32, tag=f"n_{c}_{s}")
                    nc.scalar.activation(n[:], t2[:], AF.Tanh,
                                         bias=bxn[c][:, t : t + 1])
                    # h' = n + z*(h-n); the sub runs on GPSIMD to unload DVE
                    dt_ = wp.tile([128, SW], F32, tag=f"d_{c}_{s}")
                    nc.gpsimd.tensor_sub(dt_[:], h[s][c][:], n[:])
                    e = wp.tile([128, SW], F32, tag=f"e_{c}_{s}")
                    nc.vector.tensor_mul(e[:], sig[s][2 + c][:], dt_[:])
                    hc = wp.tile([128, SW], F32, tag=f"h{c}_{s}")
                    nc.vector.tensor_add(hc[:], e[:], n[:])
                    h_new[s][c] = hc
            h = h_new

            # pred = W_out @ h' + b_out  -> DRAM out[t], and rhs for next xn
            for s in range(ns):
                bs = slice(s * SW, (s + 1) * SW)
                p = ppr.tile([OUT_DIM, SW], F32, tag="pp")
                nc.tensor.matmul(p[:], wout[0][:, :], h[s][0][:],
                                 start=True, stop=False)
                nc.tensor.matmul(p[:], wout[1][:, :], h[s][1][:],
                                 start=False, stop=True)
                pr = wp.tile([OUT_DIM, SW], F32, tag=f"pred_{s}")
                nc.scalar.activation(pr[:], p[:], AF.Identity, bias=bout[:, 0:1])
                pred[s] = pr
                nc.sync.dma_start(out[t][:, bs], pr[:])


_CACHE = {}


def _prep_host(z0, tps_to_pred, W_z0, b_z0, W_ih, b_ih, W_hh, b_hh, W_out, b_out):
    f = np.float32
    z0 = np.asarray(z0, f)
    tps = np.asarray(tps_to_pred, f)
    W_z0, b_z0 = np.asarray(W_z0, f), np.asarray(b_z0, f)
    W_ih, b_ih = np.asarray(W_ih, f), np.asarray(b_ih, f)
    W_hh, b_hh = np.asarray(W_hh, f), np.asarray(b_hh, f)
    W_out, b_out = np.asarray(W_out, f), np.asarray(b_out, f)

    Wihp = W_ih[:, :OUT_DIM]  # [768, 64]
    wt = W_ih[:, OUT_DIM]  # [768]
    G2 = 2 * N_GRU
    Weff_rz = W_hh[:G2] + Wihp[:G2] @ W_out  # [512, 256]
    whht1 = np.ascontiguousarray(
        np.concatenate([Weff_rz, W_hh[G2:]], axis=0).T
    )  # [256, 768]
    whht0 = np.ascontiguousarray(W_hh[:G2].T)  # [256, 512]
    wxnt = np.ascontiguousarray(Wihp[G2:].T)  # [64, 256]
    woutt = np.ascontiguousarray(W_out.T)  # [256, 64]

    cb = Wihp @ b_out  # [768]
    bias_all = b_ih[:, None] + wt[:, None] * tps[None, :]  # [768, 200]
    brz = bias_all[:G2] + b_hh[:G2, None]
    brz[:, 1:] += cb[:G2, None]
    bxn = bias_all[G2:].copy()
    bxn[:, 1:] += cb[G2:, None]

    shared = {
        "wz0t": np.ascontiguousarray(W_z0.T),
        "whht1": whht1,
        "whht0": whht0,
        "wxnt": wxnt,
        "woutt": woutt,
        "brz": np.ascontiguousarray(brz, f),
        "bxn": np.ascontiguousarray(bxn, f),
        "bhhn": np.ascontiguousarray(b_hh[G2:].reshape(N_GRU, 1)),
        "bz0": np.ascontiguousarray(b_z0.reshape(N_GRU, 1)),
        "bout": np.ascontiguousarray(b_out.reshape(OUT_DIM, 1)),
    }
    z0f = z0.reshape(B_FULL, LATENT)
    in_maps = []
    for i in range(N_CORES):
        m = dict(shared)
        m["z0t"] = np.ascontiguousarray(z0f[i * B_LOC : (i + 1) * B_LOC].T)
        in_maps.append(m)
    return in_maps


def _run(in_maps, repeat=1, **spmd_kwargs):
    key = f"nc{repeat}"
    if key not in _CACHE:
        _CACHE[key] = _build_module(repeat)
    return run_bass_kernel_spmd(_CACHE[key], in_maps, list(range(N_CORES)), **spmd_kwargs)


def _gather(res):
    outp = np.empty((B_FULL, N_TP, OUT_DIM), np.float32)
    for i in range(N_CORES):
        o = res.results[i]["out"]  # [200, 64, 1024]
        outp[i * B_LOC : (i + 1) * B_LOC] = np.asarray(o).transpose(2, 0, 1)
    return outp.reshape(64, 128, N_TP, OUT_DIM)


def kernel(**inputs):
    in_maps = _prep_host(**inputs)
    res = _run(in_maps)
    return _gather(res)


def kernel_profiled(**inputs):
    """Like kernel(), but requests an NTFF trace; returns (output, results)."""
    in_maps = _prep_host(**inputs)
    res = _run(in_maps, trace=True)
    return _gather(res), res
